# revision 12
# baseline (speedup 1.0000x reference)
"""Distributed Trainium2 Bass kernel for a single attention head.

Reference computation (fp32 jax):
    q = queries @ Wq.T + bq        # [B,S,Df]
    k = keys    @ Wk.T + bk
    v = values  @ Wv.T + bv
    attn = softmax((q @ k.T) / sqrt(Df), axis=-1)
    out  = attn @ v                # [B,S,Df]

with B=4, S=4096, D_MODEL=1024, D_FEATURE=64.

Sharding: 8 cores = (batch b in 0..3) x (query-half h in 0..1).
Core c handles batch b=c//2, q rows [h*2048, (h+1)*2048), with the FULL
keys/values of its batch (no collectives); host pre-transposes inputs
to m-contraction-major bf16.

Per-core kernel structure (v3):
  - Input DMAs are 2MB pair-merged (a dma_start costs ~2.5us of queue
    issue time regardless of size, so few big transfers >> many 1MB
    ones) and ordered so k pairs + q land first: the exp stream on the
    ACT engine is the kernel bottleneck (64 x [128,1024] exps ~73us).
  - k/v projections col-packed in block pairs: even block -> psE[0:64]
    (array cols 0-63), odd -> psO[64:128] (tile_position (0,64)). kT
    lands SPLIT: partitions 0-63 = even-block features, 64-127 = odd --
    the layout score pairs need, so k needs no dup. q projects with
    [wq|wq] dup'd weights (M=128).
  - scores for chunk-pair (t,c) x i-slice p (1024 wide): 4 matmuls
    interleaved E/O across TWO separate [128,1024] psum tiles so
    adjacent matmuls hit different tiles and row groups (0-63 vs
    64-127) and stream concurrently. One exp per tile.
  - attn@v: single-stream [65, 1024] psum accumulator (lhsT = v_aug
    [128, 65] with a ones column -> row 64 = softmax denominator).
  - i processed in 2 passes of 1024 q rows; kT/qT/v_sb resident; exp
    outputs park in an SBUF ring so ACT never stalls while the attn
    pass chain drains. Score tiles stream t-major while k pairs arrive,
    then pass-major so attn(1) tracks the exp tail.
  - PSUM: ss 2x[128,1024]f32 (4 banks) + attn [65,1024]f32 (2) + work
    pool (2) = 8 banks.
  - Output is written partition-major ([128, 16, 64] per core, one
    contiguous 4KB row per partition); the host untangles it for free.
"""

import numpy as np
import ml_dtypes

import concourse.bass as bass
import concourse.mybir as mybir
import concourse.tile as tile
from concourse import bacc
from concourse.bass_utils import run_bass_kernel_spmd
from concourse.masks import make_identity

B = 4
S = 4096
DM = 1024
DF = 64
NCORES = 8
SQ = S // 2          # local q rows per core
MC = DM // 128       # 8 contraction chunks
NI = 512             # proj block width
NBQ = SQ // NI       # 4 q blocks
NBK = S // NI        # 8 k/v blocks
NT = NBK // 2        # 4 k/v block-pairs
JC = S // 128        # 32 key chunks of 128
IP = 1024            # i rows per attention pass
NP = SQ // IP        # 2 passes
WB = 4 * DF          # per-m-chunk weight cols: [wq|wq|wk|wv] = 256
BLK = MC * NI        # one 512-row block: 4096 cols
BF16 = mybir.dt.bfloat16
F32 = mybir.dt.float32
NP_BF16 = ml_dtypes.bfloat16
EXP = mybir.ActivationFunctionType.Exp

# merged 2MB input DMAs (block pairs), k+q first: they gate the exp
# stream; v trails (attn drains from the parked-exp ring late).
DMA_ORDER = [
    ("k", 0), ("q", 0), ("q", 1), ("k", 1), ("v", 0),
    ("k", 2), ("v", 1), ("k", 3), ("v", 2), ("v", 3),
]

# score-round order (t = k/v block-pair, p = i-slice pass): t-major
# while DMA supplies k pairs, then pass-major so the serial attn pass
# chain tracks the exp stream tail.
TP_ORDER = [(0, 0), (0, 1), (1, 0), (1, 1), (2, 0), (3, 0), (2, 1), (3, 1)]


def build_kernel(tc):
    nc = tc.nc
    xq = nc.dram_tensor("xq", [128, NBQ * BLK], BF16, kind="ExternalInput")
    xk = nc.dram_tensor("xk", [128, NBK * BLK], BF16, kind="ExternalInput")
    xv = nc.dram_tensor("xv", [128, NBK * BLK], BF16, kind="ExternalInput")
    wT = nc.dram_tensor("wT", [128, MC * WB], BF16, kind="ExternalInput")
    bias = nc.dram_tensor("bias", [128, 3], F32, kind="ExternalInput")
    # partition-major output: row (pass*1024 + c*128 + p) lives at
    # out[p, (pass*8 + c)*64 : +64]; host reorders.
    out = nc.dram_tensor("out", [128, NP * 8 * DF], F32, kind="ExternalOutput")

    from contextlib import ExitStack

    with ExitStack() as ctx:
        const_pool = ctx.enter_context(tc.tile_pool(name="const", bufs=1))
        xin_pool = ctx.enter_context(tc.tile_pool(name="xin", bufs=5))
        act_pool = ctx.enter_context(tc.tile_pool(name="act", bufs=1))
        vtmp_pool = ctx.enter_context(tc.tile_pool(name="vtmp", bufs=2))
        pt_pool = ctx.enter_context(tc.tile_pool(name="pt", bufs=28))
        outT_pool = ctx.enter_context(tc.tile_pool(name="outT", bufs=2))
        ob_pool = ctx.enter_context(tc.tile_pool(name="ob", bufs=2))
        rcp_pool = ctx.enter_context(tc.tile_pool(name="rcp", bufs=4))
        # PSUM (8 banks): ssE/ssO [128,1024]f32 = 4, attn [65,1024]f32
        # = 2, work pool (proj/vtrans/finals) 2x2KB = 2.
        spsum = ctx.enter_context(tc.tile_pool(name="spsum", bufs=2, space="PSUM"))
        opsum = ctx.enter_context(tc.tile_pool(name="opsum", bufs=1, space="PSUM"))
        wpsum = ctx.enter_context(tc.tile_pool(name="wpsum", bufs=2, space="PSUM"))

        # ---- constants ----
        wT_sb = const_pool.tile([128, MC * WB], BF16, tag="wt")
        nc.sync.dma_start(wT_sb[:], wT[:])
        bias_sb = const_pool.tile([128, 3], F32, tag="bias")
        nc.scalar.dma_start(bias_sb[:], bias[:])
        # preload the ACT exp table while DMAs stream
        scratch = const_pool.tile([DF, 1], F32, tag="scratch")
        nc.scalar.activation(scratch[:], bias_sb[0:DF, 0:1], EXP)
        ident = const_pool.tile([128, 128], BF16, tag="ident")
        make_identity(nc, ident[:])

        # ---- PE warm-up: open the HAM clock gate before real work ----
        warm = wpsum.tile([DF, 128], F32, tag="ps")
        for _ in range(32):
            nc.tensor.matmul(warm[:], ident[:, 0:DF], ident[:], start=True, stop=True)

        # ---- merged input DMAs ----
        xmap = {"q": xq, "k": xk, "v": xv}
        pairs = {}
        for kind, i in DMA_ORDER:
            t = xin_pool.tile([128, 2 * BLK], BF16, tag="xin")
            # q pairs issue on the scalar queue: both HWDGE queues issue
            # in parallel, halving time-to-first-score
            eng = nc.scalar if kind == "q" else nc.sync
            eng.dma_start(t[:], xmap[kind][:, i * 2 * BLK:(i + 1) * 2 * BLK])
            pairs[(kind, i)] = t

        identf = const_pool.tile([128, 128], F32, tag="identf")
        make_identity(nc, identf[:])

        # ---- persistent activations ----
        qT_sb = act_pool.tile([128, SQ], BF16, tag="qT")   # dup'd halves
        kT_sb = act_pool.tile([128, SQ], BF16, tag="kT")   # split even/odd blocks
        v_sb = act_pool.tile([128, JC * (DF + 1)], BF16, tag="v")
        nc.gpsimd.memset(v_sb[:], 1.0)  # col DF of every chunk stays 1.0

        def qproj(p):
            """q block p (from pair tile (q, p//2)) -> qT_sb[:, p*512:+512]."""
            ps = wpsum.tile([128, NI], F32, tag="ps")
            x = pairs[("q", p // 2)]
            xo = (p % 2) * BLK
            for mc in range(MC):
                nc.tensor.matmul(
                    ps[:], wT_sb[:, mc * WB:mc * WB + 128],
                    x[:, xo + mc * NI:xo + (mc + 1) * NI],
                    start=(mc == 0), stop=(mc == MC - 1))
            nc.vector.tensor_scalar_add(
                qT_sb[:, p * NI:(p + 1) * NI], ps[:], bias_sb[0:128, 0:1])

        def kvproj(t, which):
            """Col-packed pair (2t, 2t+1): even block -> psE[0:64]
            (cols 0-63), odd -> psO[64:128] (tile_position (0,64))."""
            kind = "k" if which == 1 else "v"
            wofs = 128 + (which - 1) * DF
            psE = wpsum.tile([128, NI], F32, tag="ps", name="psE")
            psO = wpsum.tile([128, NI], F32, tag="ps", name="psO")
            x = pairs[(kind, t)]
            for mc in range(MC):
                w = wT_sb[:, mc * WB + wofs:mc * WB + wofs + DF]
                nc.tensor.matmul(
                    psE[0:DF, :], w, x[:, mc * NI:(mc + 1) * NI],
                    start=(mc == 0), stop=(mc == MC - 1))
                nc.tensor.matmul(
                    psO[DF:128, :], w, x[:, BLK + mc * NI:BLK + (mc + 1) * NI],
                    start=(mc == 0), stop=(mc == MC - 1))
            dst = None
            if which == 1:
                de = kT_sb[0:DF, t * NI:(t + 1) * NI]
                do = kT_sb[DF:128, t * NI:(t + 1) * NI]
                bcol = 1
            else:
                dst = vtmp_pool.tile([128, NI], BF16, tag="vtmp", name="vtmp")
                de, do = dst[0:DF, :], dst[DF:128, :]
                bcol = 2
            nc.vector.tensor_scalar_add(
                de, psE[0:DF, :], bias_sb[0:DF, bcol:bcol + 1])
            nc.vector.tensor_scalar_add(
                do, psO[DF:128, :], bias_sb[DF:128, bcol:bcol + 1])
            return dst

        def vtrans(t, vtmp):
            """[128,128] PE transposes: chunk c yields v rows for jc
            8t+c (cols 0:64) and 8t+4+c (cols 64:128)."""
            for c in range(4):
                pv = wpsum.tile([128, 128], BF16, tag="ps")
                nc.tensor.transpose(
                    pv[:], vtmp[:, c * 128:(c + 1) * 128], ident[:])
                je, jo = 8 * t + c, 8 * t + 4 + c
                nc.vector.tensor_copy(
                    v_sb[:, je * (DF + 1):je * (DF + 1) + DF], pv[:, 0:DF])
                nc.vector.tensor_copy(
                    v_sb[:, jo * (DF + 1):jo * (DF + 1) + DF], pv[:, DF:128])

        pts = {}

        def sc(t, p, c):
            """Scores + exp for chunk-pair (t,c), i-slice p (1024 wide).
            E/O interleaved across two tiles: adjacent matmuls target
            different psum tiles + row groups -> concurrent."""
            ssE = spsum.tile([128, IP], F32, tag="ss", name="ssE")
            ssO = spsum.tile([128, IP], F32, tag="ss", name="ssO")
            col = t * NI + c * 128
            io = p * IP
            for h in range(2):
                nc.tensor.matmul(
                    ssE[:, h * NI:(h + 1) * NI], kT_sb[0:DF, col:col + 128],
                    qT_sb[0:DF, io + h * NI:io + (h + 1) * NI],
                    start=True, stop=True)
            ptE = pt_pool.tile([128, IP], BF16, tag="pt", name="ptE")
            nc.scalar.activation(ptE[:], ssE[:], EXP, scale=0.125)
            for h in range(2):
                nc.tensor.matmul(
                    ssO[:, h * NI:(h + 1) * NI], kT_sb[DF:128, col:col + 128],
                    qT_sb[DF:128, io + h * NI:io + (h + 1) * NI],
                    start=True, stop=True)
            ptO = pt_pool.tile([128, IP], BF16, tag="pt", name="ptO")
            nc.scalar.activation(ptO[:], ssO[:], EXP, scale=0.125)
            pts[(t, p, c)] = (ptE, ptO)

        cur = {}

        def pass_begin():
            cur["o"] = opsum.tile([DF + 1, IP], F32, tag="po", name="poA")

        def at_piece(p, t, c):
            """attn@v: chunks 8t+c (from ptE) and 8t+4+c (ptO) into the
            [65,1024] accumulator; ones column -> row 64 = denom."""
            po = cur["o"]
            ptE, ptO = pts[(t, p, c)]
            je, jo = 8 * t + c, 8 * t + 4 + c
            first = (t == 0 and c == 0)
            last = (t == NT - 1 and c == 3)
            for h in range(2):
                nc.tensor.matmul(
                    po[:, h * NI:(h + 1) * NI],
                    v_sb[:, je * (DF + 1):(je + 1) * (DF + 1)],
                    ptE[:, h * NI:(h + 1) * NI], start=first, stop=False)
                nc.tensor.matmul(
                    po[:, h * NI:(h + 1) * NI],
                    v_sb[:, jo * (DF + 1):(jo + 1) * (DF + 1)],
                    ptO[:, h * NI:(h + 1) * NI], start=False, stop=last)

        outTs = {}

        def ev(p):
            oT = outT_pool.tile([DF + 1, IP], F32, tag="ot")
            nc.vector.tensor_copy(oT[:], cur["o"][:])
            outTs[p] = oT

        def fin(p):
            oT = outTs[p]
            ob = ob_pool.tile([128, 8, DF], F32, tag="ob")
            for c in range(8):
                pf = wpsum.tile([128, DF + 1], F32, tag="ps")
                nc.tensor.transpose(
                    pf[:], oT[:, c * 128:(c + 1) * 128],
                    identf[0:DF + 1, 0:DF + 1])
                rcp = rcp_pool.tile([128, 1], F32, tag="rcp")
                nc.vector.reciprocal(rcp[:], pf[:, DF:DF + 1])
                nc.vector.tensor_scalar_mul(ob[:, c, :], pf[:, 0:DF], rcp[:])
            nc.sync.dma_start(
                out[:, p * 8 * DF:(p + 1) * 8 * DF].rearrange(
                    "p (c f) -> p c f", f=DF),
                ob[:])

        # ---- emission schedule ----
        # Score rounds stream in TP_ORDER at ~ACT pace (ss ring
        # backpressure). A work queue of v-projections / attn pieces /
        # pass evictions / finals drains between rounds, gated on (a)
        # the piece's exp round being emitted >= LAG rounds back, (b)
        # v-projection DMA-arrival slots, (c) implicit pass chaining.
        sc_order = [(t, p, c) for (t, p) in TP_ORDER for c in range(4)]
        sc_pos = {tpc: i for i, tpc in enumerate(sc_order)}
        LAG = 1

        work = []
        for p in range(NP):
            for t in range(NT):
                if p == 0:
                    work.append(("vp", t))
                for c in range(4):
                    work.append(("at", p, t, c))
            work.append(("ev", p))
            work.append(("fin", p))
        # v pair t arrives ~{26,33,40,44}us; ACT clock ~= 12us + n*2.3us
        vp_gate = {0: 4, 1: 7, 2: 10, 3: 12}

        wi = 0

        def eligible(item, n_now):
            kind = item[0]
            if kind == "vp":
                return n_now >= vp_gate[item[1]]
            if kind == "at":
                _, p, t, c = item
                return sc_pos[(t, p, c)] + LAG <= n_now
            return True  # ev / fin

        def drain(n_now, budget):
            nonlocal wi
            done = 0
            while wi < len(work) and done < budget and eligible(work[wi], n_now):
                item = work[wi]
                if item[0] == "vp":
                    t = item[1]
                    vtrans(t, kvproj(t, 2))
                elif item[0] == "at":
                    _, p, t, c = item
                    if t == 0 and c == 0:
                        pass_begin()
                    at_piece(p, t, c)
                elif item[0] == "ev":
                    ev(item[1])
                else:
                    fin(item[1])
                wi += 1
                done += 1

        kvproj(0, 1)
        qproj(0)
        qproj(1)
        for n, (t, p, c) in enumerate(sc_order):
            if t == 0 and c == 0 and p > 0:
                qproj(2 * p)
                qproj(2 * p + 1)
            if p == 0 and c == 0 and t > 0:
                kvproj(t, 1)
            sc(t, p, c)
            drain(n, 2 if n < 16 else 3)
        drain(10 ** 9, 10 ** 9)


_COMPILED = None


def get_compiled():
    global _COMPILED
    if _COMPILED is None:
        nc = bacc.Bacc("TRN2", target_bir_lowering=False, debug=False,
                       enable_asserts=False, num_devices=NCORES)
        with tile.TileContext(nc) as tc:
            build_kernel(tc)
        nc.compile()
        _COMPILED = nc
    return _COMPILED


def _to_block_major(xT):
    """[DM, s_len] -> [128, nblk*MC*NI]: 512-col blocks, m-chunk-major inside."""
    s_len = xT.shape[1]
    nblk = s_len // NI
    return np.ascontiguousarray(
        xT.reshape(MC, 128, nblk, NI).transpose(1, 2, 0, 3).reshape(128, nblk * MC * NI))


def make_in_maps(queries, keys, values, Wq, bq, Wk, bk, Wv, bv):
    queries = np.asarray(queries, dtype=np.float32)
    keys = np.asarray(keys, dtype=np.float32)
    values = np.asarray(values, dtype=np.float32)
    WqT, WkT, WvT = np.asarray(Wq).T, np.asarray(Wk).T, np.asarray(Wv).T
    wT_full = np.concatenate([WqT, WqT, WkT, WvT], axis=1)  # [DM, 256]
    wT_host = np.ascontiguousarray(
        wT_full.reshape(MC, 128, WB).transpose(1, 0, 2).reshape(128, MC * WB)
    ).astype(NP_BF16)
    bias64 = np.stack(
        [np.asarray(bq), np.asarray(bk), np.asarray(bv)], axis=1
    ).astype(np.float32)
    bias_host = np.concatenate([bias64, bias64], axis=0)  # [128, 3]

    in_maps = []
    for c in range(NCORES):
        b, h = c // 2, c % 2
        in_maps.append({
            "xq": _to_block_major(queries[b, h * SQ:(h + 1) * SQ, :].T).astype(NP_BF16),
            "xk": _to_block_major(keys[b].T).astype(NP_BF16),
            "xv": _to_block_major(values[b].T).astype(NP_BF16),
            "wT": wT_host, "bias": bias_host,
        })
    return in_maps


def assemble(results):
    out = np.zeros((B, S, DF), dtype=np.float32)
    for c in range(NCORES):
        b, h = c // 2, c % 2
        # out dram is [128, NP*8*64]: row (pass*1024 + cc*128 + p) at
        # [p, (pass*8 + cc)*64 : +64]
        oc = results[c]["out"].reshape(128, NP, 8, DF)
        out[b, h * SQ:(h + 1) * SQ, :] = (
            oc.transpose(1, 2, 0, 3).reshape(SQ, DF))
    return out


def kernel(**inputs):
    nc = get_compiled()
    in_maps = make_in_maps(**inputs)
    res = run_bass_kernel_spmd(nc, in_maps, core_ids=list(range(NCORES)))
    return assemble(res.results)


# revision 13
# speedup vs baseline: 1.2547x; 1.2547x over previous
"""Distributed Trainium2 Bass kernel for a single attention head.

Reference computation (fp32 jax):
    q = queries @ Wq.T + bq        # [B,S,Df]
    k = keys    @ Wk.T + bk
    v = values  @ Wv.T + bv
    attn = softmax((q @ k.T) / sqrt(Df), axis=-1)
    out  = attn @ v                # [B,S,Df]

with B=4, S=4096, D_MODEL=1024, D_FEATURE=64.

Sharding: 8 cores = (batch b in 0..3) x (query-half h in 0..1).
Core c handles batch b=c//2, q rows [h*2048, (h+1)*2048), with the FULL
keys/values of its batch (no collectives); host pre-transposes inputs
to m-contraction-major bf16.

Per-core kernel structure (v3):
  - Input DMAs are 2MB pair-merged (a dma_start costs ~2.5us of queue
    issue time regardless of size, so few big transfers >> many 1MB
    ones) and ordered so k pairs + q land first: the exp stream on the
    ACT engine is the kernel bottleneck (64 x [128,1024] exps ~73us).
  - k/v projections col-packed in block pairs: even block -> psE[0:64]
    (array cols 0-63), odd -> psO[64:128] (tile_position (0,64)). kT
    lands SPLIT: partitions 0-63 = even-block features, 64-127 = odd --
    the layout score pairs need, so k needs no dup. q projects with
    [wq|wq] dup'd weights (M=128).
  - scores for chunk-pair (t,c) x i-slice p (1024 wide): 4 matmuls
    interleaved E/O across TWO separate [128,1024] psum tiles so
    adjacent matmuls hit different tiles and row groups (0-63 vs
    64-127) and stream concurrently. One exp per tile.
  - attn@v: single-stream [65, 1024] psum accumulator (lhsT = v_aug
    [128, 65] with a ones column -> row 64 = softmax denominator).
  - i processed in 2 passes of 1024 q rows; kT/qT/v_sb resident; exp
    outputs park in an SBUF ring so ACT never stalls while the attn
    pass chain drains. Score tiles stream t-major while k pairs arrive,
    then pass-major so attn(1) tracks the exp tail.
  - PSUM: ss 2x[128,1024]f32 (4 banks) + attn [65,1024]f32 (2) + work
    pool (2) = 8 banks.
  - Output is written partition-major ([128, 16, 64] per core, one
    contiguous 4KB row per partition); the host untangles it for free.
"""

import numpy as np
import ml_dtypes

import concourse.bass as bass
import concourse.mybir as mybir
import concourse.tile as tile
from concourse import bacc
from concourse.bass_utils import run_bass_kernel_spmd
from concourse.masks import make_identity

B = 4
S = 4096
DM = 1024
DF = 64
NCORES = 8
SQ = S // 2          # local q rows per core
MC = DM // 128       # 8 contraction chunks
NI = 512             # proj block width
NBQ = SQ // NI       # 4 q blocks
NBK = S // NI        # 8 k/v blocks
NT = NBK // 2        # 4 k/v block-pairs
JC = S // 128        # 32 key chunks of 128
IP = 1024            # i rows per attention pass
NP = SQ // IP        # 2 passes
WB = 4 * DF          # per-m-chunk weight cols: [wq|wq|wk|wv] = 256
BLK = MC * NI        # one 512-row block: 4096 cols
BF16 = mybir.dt.bfloat16
F32 = mybir.dt.float32
NP_BF16 = ml_dtypes.bfloat16
EXP = mybir.ActivationFunctionType.Exp

# merged 2MB input DMAs (block pairs), k+q first: they gate the exp
# stream; v trails (attn drains from the parked-exp ring late).
DMA_ORDER = [
    ("k", 0), ("q", 0), ("q", 1), ("k", 1), ("v", 0),
    ("k", 2), ("v", 1), ("k", 3), ("v", 2), ("v", 3),
]

# score-round order (t = k/v block-pair, p = i-slice pass): t-major
# while DMA supplies k pairs, then pass-major so the serial attn pass
# chain tracks the exp stream tail.
TP_ORDER = [(0, 0), (0, 1), (1, 0), (1, 1), (2, 0), (3, 0), (2, 1), (3, 1)]


def build_kernel(tc):
    nc = tc.nc
    xq = nc.dram_tensor("xq", [128, NBQ * BLK], BF16, kind="ExternalInput")
    xk = nc.dram_tensor("xk", [128, NBK * BLK], BF16, kind="ExternalInput")
    xv = nc.dram_tensor("xv", [128, NBK * BLK], BF16, kind="ExternalInput")
    wT = nc.dram_tensor("wT", [128, MC * WB], BF16, kind="ExternalInput")
    bias = nc.dram_tensor("bias", [128, 3], F32, kind="ExternalInput")
    # partition-major output: row (pass*1024 + c*128 + p) lives at
    # out[p, (pass*8 + c)*64 : +64]; host reorders.
    out = nc.dram_tensor("out", [128, NP * 8 * DF], F32, kind="ExternalOutput")

    from contextlib import ExitStack

    with ExitStack() as ctx:
        const_pool = ctx.enter_context(tc.tile_pool(name="const", bufs=1))
        xin_pool = ctx.enter_context(tc.tile_pool(name="xin", bufs=5))
        act_pool = ctx.enter_context(tc.tile_pool(name="act", bufs=1))
        vtmp_pool = ctx.enter_context(tc.tile_pool(name="vtmp", bufs=2))
        pt_pool = ctx.enter_context(tc.tile_pool(name="pt", bufs=28))
        outT_pool = ctx.enter_context(tc.tile_pool(name="outT", bufs=2))
        ob_pool = ctx.enter_context(tc.tile_pool(name="ob", bufs=2))
        rcp_pool = ctx.enter_context(tc.tile_pool(name="rcp", bufs=4))
        # PSUM (8 banks): ssE/ssO [128,1024]f32 = 4, attn [65,1024]f32
        # = 2, work pool (proj/vtrans/finals) 2x2KB = 2.
        spsum = ctx.enter_context(tc.tile_pool(name="spsum", bufs=2, space="PSUM"))
        opsum = ctx.enter_context(tc.tile_pool(name="opsum", bufs=1, space="PSUM"))
        wpsum = ctx.enter_context(tc.tile_pool(name="wpsum", bufs=2, space="PSUM"))

        # ---- constants ----
        wT_sb = const_pool.tile([128, MC * WB], BF16, tag="wt")
        nc.sync.dma_start(wT_sb[:], wT[:])
        bias_sb = const_pool.tile([128, 3], F32, tag="bias")
        nc.scalar.dma_start(bias_sb[:], bias[:])
        # preload the ACT exp table while DMAs stream
        scratch = const_pool.tile([DF, 1], F32, tag="scratch")
        nc.scalar.activation(scratch[:], bias_sb[0:DF, 0:1], EXP)
        ident = const_pool.tile([128, 128], BF16, tag="ident")
        make_identity(nc, ident[:])

        # ---- PE warm-up: open the HAM clock gate before real work ----
        warm = wpsum.tile([DF, 128], F32, tag="ps")
        for _ in range(32):
            nc.tensor.matmul(warm[:], ident[:, 0:DF], ident[:], start=True, stop=True)

        # ---- merged input DMAs ----
        xmap = {"q": xq, "k": xk, "v": xv}
        pairs = {}
        for kind, i in DMA_ORDER:
            t = xin_pool.tile([128, 2 * BLK], BF16, tag="xin")
            # q pairs issue on the scalar queue: both HWDGE queues issue
            # in parallel, halving time-to-first-score
            eng = nc.scalar if kind == "q" else nc.sync
            eng.dma_start(t[:], xmap[kind][:, i * 2 * BLK:(i + 1) * 2 * BLK])
            pairs[(kind, i)] = t

        identf = const_pool.tile([128, 128], F32, tag="identf")
        make_identity(nc, identf[:])

        # ---- persistent activations ----
        qT_sb = act_pool.tile([128, SQ], BF16, tag="qT")   # dup'd halves
        kT_sb = act_pool.tile([128, SQ], BF16, tag="kT")   # split even/odd blocks
        v_sb = act_pool.tile([128, JC * (DF + 1)], BF16, tag="v")
        nc.gpsimd.memset(v_sb[:], 1.0)  # col DF of every chunk stays 1.0

        def qproj(p):
            """q block p (from pair tile (q, p//2)) -> qT_sb[:, p*512:+512]."""
            ps = wpsum.tile([128, NI], F32, tag="ps")
            x = pairs[("q", p // 2)]
            xo = (p % 2) * BLK
            for mc in range(MC):
                nc.tensor.matmul(
                    ps[:], wT_sb[:, mc * WB:mc * WB + 128],
                    x[:, xo + mc * NI:xo + (mc + 1) * NI],
                    start=(mc == 0), stop=(mc == MC - 1))
            nc.vector.tensor_scalar_add(
                qT_sb[:, p * NI:(p + 1) * NI], ps[:], bias_sb[0:128, 0:1])

        def kvproj(t, which):
            """Col-packed pair (2t, 2t+1): even block -> psE[0:64]
            (cols 0-63), odd -> psO[64:128] (tile_position (0,64))."""
            kind = "k" if which == 1 else "v"
            wofs = 128 + (which - 1) * DF
            psE = wpsum.tile([128, NI], F32, tag="ps", name="psE")
            psO = wpsum.tile([128, NI], F32, tag="ps", name="psO")
            x = pairs[(kind, t)]
            for mc in range(MC):
                w = wT_sb[:, mc * WB + wofs:mc * WB + wofs + DF]
                nc.tensor.matmul(
                    psE[0:DF, :], w, x[:, mc * NI:(mc + 1) * NI],
                    start=(mc == 0), stop=(mc == MC - 1))
                nc.tensor.matmul(
                    psO[DF:128, :], w, x[:, BLK + mc * NI:BLK + (mc + 1) * NI],
                    start=(mc == 0), stop=(mc == MC - 1))
            dst = None
            if which == 1:
                de = kT_sb[0:DF, t * NI:(t + 1) * NI]
                do = kT_sb[DF:128, t * NI:(t + 1) * NI]
                bcol = 1
            else:
                dst = vtmp_pool.tile([128, NI], BF16, tag="vtmp", name="vtmp")
                de, do = dst[0:DF, :], dst[DF:128, :]
                bcol = 2
            nc.vector.tensor_scalar_add(
                de, psE[0:DF, :], bias_sb[0:DF, bcol:bcol + 1])
            nc.vector.tensor_scalar_add(
                do, psO[DF:128, :], bias_sb[DF:128, bcol:bcol + 1])
            return dst

        def vtrans(t, vtmp):
            """[128,128] PE transposes: chunk c yields v rows for jc
            8t+c (cols 0:64) and 8t+4+c (cols 64:128)."""
            for c in range(4):
                pv = wpsum.tile([128, 128], BF16, tag="ps")
                nc.tensor.transpose(
                    pv[:], vtmp[:, c * 128:(c + 1) * 128], ident[:])
                je, jo = 8 * t + c, 8 * t + 4 + c
                nc.vector.tensor_copy(
                    v_sb[:, je * (DF + 1):je * (DF + 1) + DF], pv[:, 0:DF])
                nc.vector.tensor_copy(
                    v_sb[:, jo * (DF + 1):jo * (DF + 1) + DF], pv[:, DF:128])

        pts = {}

        def sc(t, p, c):
            """Scores + exp for chunk-pair (t,c), i-slice p (1024 wide).
            E/O interleaved across two tiles: adjacent matmuls target
            different psum tiles + row groups -> concurrent."""
            ssE = spsum.tile([128, IP], F32, tag="ss", name="ssE")
            ssO = spsum.tile([128, IP], F32, tag="ss", name="ssO")
            col = t * NI + c * 128
            io = p * IP
            for h in range(2):
                nc.tensor.matmul(
                    ssE[:, h * NI:(h + 1) * NI], kT_sb[0:DF, col:col + 128],
                    qT_sb[0:DF, io + h * NI:io + (h + 1) * NI],
                    start=True, stop=True)
            ptE = pt_pool.tile([128, IP], BF16, tag="pt", name="ptE")
            nc.scalar.activation(ptE[:], ssE[:], EXP, scale=0.125)
            for h in range(2):
                nc.tensor.matmul(
                    ssO[:, h * NI:(h + 1) * NI], kT_sb[DF:128, col:col + 128],
                    qT_sb[DF:128, io + h * NI:io + (h + 1) * NI],
                    start=True, stop=True)
            ptO = pt_pool.tile([128, IP], BF16, tag="pt", name="ptO")
            nc.scalar.activation(ptO[:], ssO[:], EXP, scale=0.125)
            pts[(t, p, c)] = (ptE, ptO)

        cur = {}

        def pass_begin():
            cur["o"] = opsum.tile([DF + 1, IP], F32, tag="po", name="poA")

        def at_piece(p, t, c):
            """attn@v: chunks 8t+c (from ptE) and 8t+4+c (ptO) into the
            [65,1024] accumulator; ones column -> row 64 = denom."""
            po = cur["o"]
            ptE, ptO = pts[(t, p, c)]
            je, jo = 8 * t + c, 8 * t + 4 + c
            first = (t == 0 and c == 0)
            last = (t == NT - 1 and c == 3)
            for h in range(2):
                nc.tensor.matmul(
                    po[:, h * NI:(h + 1) * NI],
                    v_sb[:, je * (DF + 1):(je + 1) * (DF + 1)],
                    ptE[:, h * NI:(h + 1) * NI], start=first, stop=False)
                nc.tensor.matmul(
                    po[:, h * NI:(h + 1) * NI],
                    v_sb[:, jo * (DF + 1):(jo + 1) * (DF + 1)],
                    ptO[:, h * NI:(h + 1) * NI], start=False, stop=last)

        outTs = {}

        def ev(p):
            oT = outT_pool.tile([DF + 1, IP], F32, tag="ot")
            nc.vector.tensor_copy(oT[:], cur["o"][:])
            outTs[p] = oT

        def fin(p):
            oT = outTs[p]
            ob = ob_pool.tile([128, 8, DF], F32, tag="ob")
            for c in range(8):
                pf = wpsum.tile([128, DF + 1], F32, tag="ps")
                nc.tensor.transpose(
                    pf[:], oT[:, c * 128:(c + 1) * 128],
                    identf[0:DF + 1, 0:DF + 1])
                rcp = rcp_pool.tile([128, 1], F32, tag="rcp")
                nc.vector.reciprocal(rcp[:], pf[:, DF:DF + 1])
                nc.vector.tensor_scalar_mul(ob[:, c, :], pf[:, 0:DF], rcp[:])
            nc.sync.dma_start(
                out[:, p * 8 * DF:(p + 1) * 8 * DF].rearrange(
                    "p (c f) -> p c f", f=DF),
                ob[:])

        # ---- emission schedule ----
        # Score rounds stream in TP_ORDER at ~ACT pace (ss ring
        # backpressure). A work queue of v-projections / attn pieces /
        # pass evictions / finals drains between rounds, gated on (a)
        # the piece's exp round being emitted >= LAG rounds back, (b)
        # v-projection DMA-arrival slots, (c) implicit pass chaining.
        sc_order = [(t, p, c) for (t, p) in TP_ORDER for c in range(4)]
        sc_pos = {tpc: i for i, tpc in enumerate(sc_order)}
        LAG = 1

        work = []
        for p in range(NP):
            for t in range(NT):
                if p == 0:
                    work.append(("vp", t))
                for c in range(4):
                    work.append(("at", p, t, c))
            work.append(("ev", p))
            work.append(("fin", p))
        # v pair t arrives ~{26,33,40,44}us; ACT clock ~= 12us + n*2.3us
        vp_gate = {0: 8, 1: 11, 2: 13, 3: 15}

        wi = 0

        def eligible(item, n_now):
            kind = item[0]
            if kind == "vp":
                return n_now >= vp_gate[item[1]]
            if kind == "at":
                _, p, t, c = item
                return sc_pos[(t, p, c)] + LAG <= n_now
            return True  # ev / fin

        def drain(n_now, budget):
            nonlocal wi
            done = 0
            while wi < len(work) and done < budget and eligible(work[wi], n_now):
                item = work[wi]
                if item[0] == "vp":
                    t = item[1]
                    vtrans(t, kvproj(t, 2))
                elif item[0] == "at":
                    _, p, t, c = item
                    if t == 0 and c == 0:
                        pass_begin()
                    at_piece(p, t, c)
                elif item[0] == "ev":
                    ev(item[1])
                else:
                    fin(item[1])
                wi += 1
                done += 1

        kvproj(0, 1)
        qproj(0)
        qproj(1)
        for n, (t, p, c) in enumerate(sc_order):
            if t == 0 and c == 0 and p > 0:
                qproj(2 * p)
                qproj(2 * p + 1)
            if p == 0 and c == 0 and t > 0:
                kvproj(t, 1)
            sc(t, p, c)
            drain(n, 2 if n < 16 else 3)
        drain(10 ** 9, 10 ** 9)


_COMPILED = None


def get_compiled():
    global _COMPILED
    if _COMPILED is None:
        nc = bacc.Bacc("TRN2", target_bir_lowering=False, debug=False,
                       enable_asserts=False, num_devices=NCORES)
        with tile.TileContext(nc) as tc:
            build_kernel(tc)
        nc.compile()
        _COMPILED = nc
    return _COMPILED


def _to_block_major(xT):
    """[DM, s_len] -> [128, nblk*MC*NI]: 512-col blocks, m-chunk-major inside."""
    s_len = xT.shape[1]
    nblk = s_len // NI
    return np.ascontiguousarray(
        xT.reshape(MC, 128, nblk, NI).transpose(1, 2, 0, 3).reshape(128, nblk * MC * NI))


def make_in_maps(queries, keys, values, Wq, bq, Wk, bk, Wv, bv):
    queries = np.asarray(queries, dtype=np.float32)
    keys = np.asarray(keys, dtype=np.float32)
    values = np.asarray(values, dtype=np.float32)
    WqT, WkT, WvT = np.asarray(Wq).T, np.asarray(Wk).T, np.asarray(Wv).T
    wT_full = np.concatenate([WqT, WqT, WkT, WvT], axis=1)  # [DM, 256]
    wT_host = np.ascontiguousarray(
        wT_full.reshape(MC, 128, WB).transpose(1, 0, 2).reshape(128, MC * WB)
    ).astype(NP_BF16)
    bias64 = np.stack(
        [np.asarray(bq), np.asarray(bk), np.asarray(bv)], axis=1
    ).astype(np.float32)
    bias_host = np.concatenate([bias64, bias64], axis=0)  # [128, 3]

    in_maps = []
    for c in range(NCORES):
        b, h = c // 2, c % 2
        in_maps.append({
            "xq": _to_block_major(queries[b, h * SQ:(h + 1) * SQ, :].T).astype(NP_BF16),
            "xk": _to_block_major(keys[b].T).astype(NP_BF16),
            "xv": _to_block_major(values[b].T).astype(NP_BF16),
            "wT": wT_host, "bias": bias_host,
        })
    return in_maps


def assemble(results):
    out = np.zeros((B, S, DF), dtype=np.float32)
    for c in range(NCORES):
        b, h = c // 2, c % 2
        # out dram is [128, NP*8*64]: row (pass*1024 + cc*128 + p) at
        # [p, (pass*8 + cc)*64 : +64]
        oc = results[c]["out"].reshape(128, NP, 8, DF)
        out[b, h * SQ:(h + 1) * SQ, :] = (
            oc.transpose(1, 2, 0, 3).reshape(SQ, DF))
    return out


def kernel(**inputs):
    nc = get_compiled()
    in_maps = make_in_maps(**inputs)
    res = run_bass_kernel_spmd(nc, in_maps, core_ids=list(range(NCORES)))
    return assemble(res.results)
